# revision 13
# baseline (speedup 1.0000x reference)
"""Trainium2 Bass kernel for a directed MPNN layer (8 NeuronCores, SPMD).

Reference computation (per edge e = (src, tgt)):
    msg  = relu(edge_hidden @ W_msg.T + b_msg)                     (E, H)
    agg  = segment_sum(msg, tgt, N)                                (N, H)
    excl[e] = sum msg[f] over f with (tgt_f, src_f) == (src_e, tgt_e)
    out[e]  = relu(x[src_e] @ Wx.T + edge_attr[e] @ Wa.T
                   + (agg[src_e] - excl[e]) @ Wm.T + b_upd)
  with W_upd = [Wx | Wa | Wm] split along columns (64 | 16 | 64).

Decomposition (no cross-core communication):
    nt[v]  = x[v] @ Wx.T + agg[v] @ Wm.T + b_upd
    out[e] = relu(nt[src_e] + edge_attr[e] @ Wa.T - relu(msg_f) @ Wm.T)
  for e = rev(f) (reverse pairs; rare duplicate pairs patched at the end).

Layout: each core owns 5000 nodes (40 blocks x 128), in-edges tgt-sorted
into k_blk chunks of 128 per block.  PARITY PACKING: chunk ch sits on
eh partition-half (ch%2), column (ch//2)*128 -- even/odd chunks pair up
so matmuls run col-tiled concurrently on [128, 512] tiles.

SOFTWARE PIPELINE (per iteration i):
  1. DMAs + one-hot generation for block i (t4 via DVE tensor_tensor,
     u2 via gpsimd row-broadcast + DVE tensor_scalar)
  2. pass1(i): msg MMs -> relu -> agg scatter MMs (pipelined groups)
  3. pass2(i-1): msgT via Wpair MM, Wstack + nt-gather MMs (col-tiled),
     relu -> out  -- overlaps the agg->nt joint of block i
  4. nt chain(i): one [128,128] agg copy, nt MM (Wum doubled rows; bias
     folded into the x matmul as an extra ones-row of xT), ACT cast
"""

import numpy as np
import ml_dtypes

import concourse.bacc as bacc
import concourse.bass as bass
import concourse.mybir as mybir
import concourse.tile as tile
from concourse.bass_utils import run_bass_kernel_spmd

F32 = mybir.dt.float32
F32R = mybir.dt.float32r
BF16 = mybir.dt.bfloat16
I32 = mybir.dt.int32
ALU = mybir.AluOpType
ACTF = mybir.ActivationFunctionType
NPBF = ml_dtypes.bfloat16

N = 40000
E = 800000
E2 = E // 2
H = 64
A = 16
NC = 8
P = 128

NPC = N // NC           # 5000 nodes per core
NBLK = 40               # 128-node blocks per core
NPC_PAD = NBLK * P      # 5120
SPEC_CAP = P            # special (correction) rows per core

_CACHE = {}

# packB column offsets (bf16 [P, 1024])
PB_WPAIR = 0      # [128, 128] blockdiag(W_msg.T, W_msg.T)
PB_WMSG2 = 128    # [128, 64]  W_msg.T doubled rows
PB_NWUM2 = 192    # [128, 64]  -W_upd_m.T doubled rows
PB_WSTACK = 256   # [80, 64]   [negWum ; Wua]
PB_WUA = 320      # [16, 64]
PB_IOTA1 = 384    # [128, 128] cols 0..127 on every row
PB_IDENT = 512    # [128, 128]
PB_EHF = 640      # [64, 128]
PB_EHRF = 768     # [64, 128]
PB_ATTRF = 896    # [16, 128]
PB_COLS = 1024
# packR column offsets (f32r [P, 256])
PR_WUM2 = 0       # [128, 64]  [Wum ; Wum] stacked rows
PR_WUXB = 64      # [65, 64]   [Wux ; b_upd]
PR_SNEG = 128     # [128, 128]
PR_COLS = 256
# packF column offsets (f32 [P, 2])
PF_IOTAP = 0      # [128, 1] partition index
PF_COLS = 2


def _build(k_blk: int):
    assert k_blk % 2 == 0
    nch = NBLK * k_blk              # chunks per core
    l1 = nch * P                    # padded edge slots per core
    lblk = k_blk * P                # edge slots per block
    hb = lblk // 2                  # half-block edge slots
    kp = k_blk // 2                 # chunk pairs per block

    nc = bacc.Bacc("TRN2", target_bir_lowering=False, debug=False,
                   num_devices=NC)

    def inp(name, shape, dtype):
        return nc.dram_tensor(name, shape, dtype, kind="ExternalInput").ap()

    # eh: chunk ch on partitions 64*(ch%2):, column (ch//2)*128
    eh2 = inp("eh2", [P, (nch // 2) * P], BF16)
    tgt_rel8 = inp("tgt_rel8", [P, 8 * nch], BF16)   # 8-replicated trel
    trel_bca = inp("trel_bca", [P, l1], BF16)  # trel bcast, parity order
    attr_T = inp("attr_T", [A, l1], BF16)             # parity-split order
    xT_own = inp("xT_own", [H + 1, NPC_PAD], F32R)    # last row = ones
    packB = inp("packB", [P, PB_COLS], BF16)
    packR = inp("packR", [P, PR_COLS], F32R)
    packF = inp("packF", [P, PF_COLS], F32)
    didx = inp("didx", [P, 1], I32)

    outT2 = nc.dram_tensor("outT2", [P, l1 // 2], BF16,
                           kind="ExternalOutput").ap()
    outF = nc.dram_tensor("outF", [H, P], BF16, kind="ExternalOutput").ap()
    nt_own = nc.dram_tensor("nt_own", [NPC_PAD, H], BF16).ap()

    with tile.TileContext(nc) as tc:
        with (
            tc.tile_pool(name="const", bufs=1) as cst,
            tc.tile_pool(name="ehp", bufs=3) as ehp,
            tc.tile_pool(name="sb", bufs=4) as sb,
            tc.tile_pool(name="blk", bufs=5) as blk,
            tc.tile_pool(name="sx", bufs=2) as sxp,
            tc.tile_pool(name="osb", bufs=2) as osb,
            tc.tile_pool(name="ntp", bufs=3) as ntp,
            tc.tile_pool(name="ps_msg", bufs=2, space="PSUM") as ps_msg,
            tc.tile_pool(name="ps_agg", bufs=2, space="PSUM") as ps_agg,
            tc.tile_pool(name="ps_m", bufs=2, space="PSUM") as ps_m,
            tc.tile_pool(name="ps_o", bufs=2, space="PSUM") as ps_o,
        ):
            packB_t = cst.tile([P, PB_COLS], BF16, tag="c_packB")
            nc.sync.dma_start(packB_t[:], packB[:])
            packR_t = cst.tile([P, PR_COLS], F32R, tag="c_packR")
            nc.sync.dma_start(packR_t[:], packR[:])
            packF_t = cst.tile([P, PF_COLS], F32, tag="c_packF")
            nc.sync.dma_start(packF_t[:], packF[:])
            tgt_rel8_sb = cst.tile([P, 8 * nch], BF16, tag="c_tgtrel8")
            nc.sync.dma_start(tgt_rel8_sb[:], tgt_rel8[:])
            xT_sb = cst.tile([H + 1, NPC_PAD], F32R, tag="c_xt")
            nc.sync.dma_start(xT_sb[:], xT_own[:])
            didx_sb = cst.tile([P, 1], I32, tag="c_didx")
            nc.sync.dma_start(didx_sb[:], didx[:])

            Wpair_ap = packB_t[:, PB_WPAIR:PB_WPAIR + P]
            Wmsg2_ap = packB_t[0:H, PB_WMSG2:PB_WMSG2 + H]
            negWum_ap = packB_t[0:H, PB_NWUM2:PB_NWUM2 + H]
            Wstack_ap = packB_t[0:H + A, PB_WSTACK:PB_WSTACK + H]
            Wua_ap = packB_t[0:A, PB_WUA:PB_WUA + H]
            iota1_ap = packB_t[:, PB_IOTA1:PB_IOTA1 + P]
            ident_ap = packB_t[:, PB_IDENT:PB_IDENT + P]
            ehF_ap = packB_t[0:H, PB_EHF:PB_EHF + P]
            ehRF_ap = packB_t[0:H, PB_EHRF:PB_EHRF + P]
            attrF_ap = packB_t[0:A, PB_ATTRF:PB_ATTRF + P]
            Wum2_ap = packR_t[:, PR_WUM2:PR_WUM2 + H]
            Wum_ap = packR_t[0:H, PR_WUM2:PR_WUM2 + H]
            WuxB_ap = packR_t[0:H + 1, PR_WUXB:PR_WUXB + H]
            Sneg_ap = packR_t[:, PR_SNEG:PR_SNEG + P]
            iotaP_ap = packF_t[:, PF_IOTAP:PF_IOTAP + 1]

            prev = None
            for i in range(NBLK + 1):
                # ---- stage 1: DMAs + one-hots for block i ----
                if i < NBLK:
                    b = i
                    c0 = b * k_blk
                    e0 = (c0 // 2) * P

                    ehb = ehp.tile([P, hb], BF16, tag="ehb")
                    nc.sync.dma_start(ehb[:], eh2[:, e0:e0 + hb])

                    t4 = blk.tile([P, lblk], BF16, tag="t4")
                    tr = tgt_rel8_sb[:, 8 * c0:8 * (c0 + k_blk)]
                    in1 = bass.AP(tr.tensor, tr.offset,
                                  tr.ap[:1] + [[8, k_blk], [0, 16], [1, 8]])
                    in0 = bass.AP(iota1_ap.tensor, iota1_ap.offset,
                                  iota1_ap.ap[:1]
                                  + [[0, k_blk], [8, 16], [1, 8]])
                    nc.vector.tensor_tensor(out=t4[:], in0=in0, in1=in1,
                                            op=ALU.is_equal)

                    trel_bc = blk.tile([P, lblk], BF16, tag="tbc")
                    nc.sync.dma_start(trel_bc[:],
                                      trel_bca[:, b * lblk:(b + 1) * lblk])
                    u2 = blk.tile([P, lblk], BF16, tag="u2")
                    nc.vector.tensor_scalar(out=u2[:], in0=trel_bc[:],
                                            scalar1=iotaP_ap,
                                            scalar2=None, op0=ALU.is_equal)

                    sxE = sxp.tile([H + A, hb], BF16, tag="sxE")
                    nc.sync.dma_start(sxE[H:H + A, :],
                                      attr_T[:, c0 * P:c0 * P + hb])
                    sxO = sxp.tile([H + A, hb], BF16, tag="sxO")
                    nc.sync.dma_start(sxO[H:H + A, :],
                                      attr_T[:, c0 * P + hb:c0 * P + lblk])

                    # ---- stage 2: pass1(i) msg -> relu -> scatter ----
                    agg2_ps = ps_agg.tile([P, P], F32, tag="agg")
                    groups = []
                    pr = 0
                    while pr < kp:
                        gwp = min(4, kp - pr)
                        groups.append((pr, gwp))
                        pr += gwp

                    def emit_msg(g, gi):
                        pr0, gwp = groups[g]
                        msg_ps = ps_msg.tile([P, 4 * P], F32, tag="msg")
                        for jp in range(gwp):
                            pc_ = pr0 + jp
                            nc.tensor.matmul(
                                msg_ps[:, jp * P:(jp + 1) * P],
                                lhsT=ehb[:, pc_ * P:(pc_ + 1) * P],
                                rhs=Wpair_ap, start=True, stop=True)
                        msg_sb = sb.tile([P, 4 * P], BF16, tag="msg_sb")
                        nc.scalar.activation(msg_sb[:, :gwp * P],
                                             msg_ps[:, :gwp * P],
                                             ACTF.Relu)
                        return msg_sb

                    def emit_scat(g, msg_sb):
                        pr0, gwp = groups[g]
                        for jp in range(gwp):
                            pc_ = pr0 + jp
                            nc.tensor.matmul(
                                agg2_ps[0:H, :],
                                lhsT=msg_sb[:, jp * P:jp * P + H],
                                rhs=t4[:, (2 * pc_) * P:(2 * pc_ + 1) * P],
                                start=(pc_ == 0), stop=(pc_ == kp - 1),
                                skip_group_check=True)
                            nc.tensor.matmul(
                                agg2_ps[H:P, :],
                                lhsT=msg_sb[:, jp * P + H:(jp + 1) * P],
                                rhs=t4[:, (2 * pc_ + 1) * P:
                                       (2 * pc_ + 2) * P],
                                start=(pc_ == 0), stop=(pc_ == kp - 1),
                                skip_group_check=True,
                                tile_position=(0, 64))

                    ng = len(groups)
                    msg_tiles = [None] * ng
                    msg_tiles[0] = emit_msg(0, 0)
                    if ng > 1:
                        msg_tiles[1] = emit_msg(1, 1)
                    for g in range(ng):
                        emit_scat(g, msg_tiles[g])
                        if g + 2 < ng:
                            msg_tiles[g + 2] = emit_msg(g + 2, g + 2)

                # ---- stage 3: pass2(i-1) ----
                if prev is not None:
                    (pehb, pu2, psxE, psxO, pnt_sb, pe0) = prev
                    outsb = osb.tile([P, hb], BF16, tag="outsb")
                    grp2 = []
                    pc = 0
                    while pc < kp:
                        gw2 = min(4, kp - pc)
                        grp2.append((pc, gw2))
                        pc += gw2

                    def emit_m(g2, gi2):
                        pc0, gw2 = grp2[g2]
                        w2 = gw2 * P
                        g0 = pc0 * P
                        m_ps = ps_m.tile([P, 4 * P], F32, tag="m")
                        nc.tensor.matmul(m_ps[:, 0:w2], lhsT=Wpair_ap,
                                         rhs=pehb[:, g0:g0 + w2],
                                         start=True, stop=True)
                        nc.scalar.activation(psxE[0:H, g0:g0 + w2],
                                             m_ps[0:H, 0:w2], ACTF.Relu)
                        nc.vector.tensor_scalar(
                            out=psxO[0:H, g0:g0 + w2],
                            in0=m_ps[H:P, 0:w2],
                            scalar1=0.0, scalar2=None, op0=ALU.max)

                    def emit_o(g2):
                        pc0, gw2 = grp2[g2]
                        w2 = gw2 * P
                        g0 = pc0 * P
                        o_ps = ps_o.tile([P, 4 * P], F32, tag="o")
                        nc.tensor.matmul(o_ps[0:H, 0:w2], lhsT=Wstack_ap,
                                         rhs=psxE[:, g0:g0 + w2],
                                         start=True, stop=False,
                                         skip_group_check=True)
                        nc.tensor.matmul(o_ps[0:H, 0:w2], lhsT=pnt_sb[:],
                                         rhs=pu2[:, g0:g0 + w2],
                                         start=False, stop=True,
                                         skip_group_check=True)
                        nc.tensor.matmul(o_ps[H:P, 0:w2], lhsT=Wstack_ap,
                                         rhs=psxO[:, g0:g0 + w2],
                                         start=True, stop=False,
                                         skip_group_check=True,
                                         tile_position=(0, 64))
                        nc.tensor.matmul(o_ps[H:P, 0:w2], lhsT=pnt_sb[:],
                                         rhs=pu2[:, hb + g0:hb + g0 + w2],
                                         start=False, stop=True,
                                         skip_group_check=True,
                                         tile_position=(0, 64))
                        if g2 == 1:
                            nc.scalar.activation(outsb[:, g0:g0 + w2],
                                                 o_ps[:, 0:w2], ACTF.Relu)
                        else:
                            nc.vector.tensor_scalar(
                                out=outsb[:, g0:g0 + w2],
                                in0=o_ps[:, 0:w2],
                                scalar1=0.0, scalar2=None, op0=ALU.max)

                    n2 = len(grp2)
                    emit_m(0, 0)
                    if n2 > 1:
                        emit_m(1, 1)
                    for g2 in range(n2):
                        emit_o(g2)
                        if g2 + 2 < n2:
                            emit_m(g2 + 2, g2 + 2)
                    nc.sync.dma_start(outT2[:, pe0:pe0 + hb], outsb[:])

                # ---- stage 4: nt chain(i) ----
                if i < NBLK:
                    agg_sb = sb.tile([P, P], F32R, tag="agg_sb")
                    nc.vector.tensor_copy(agg_sb[:], agg2_ps[:])
                    nt_ps = ps_msg.tile([P, H], F32, tag="msg")
                    nc.tensor.matmul(nt_ps[:], lhsT=agg_sb[:], rhs=Wum2_ap,
                                     start=True, stop=False)
                    nc.tensor.matmul(nt_ps[:],
                                     lhsT=xT_sb[:, b * P:(b + 1) * P],
                                     rhs=WuxB_ap, start=False, stop=True)
                    nt_sb = ntp.tile([P, H], BF16, tag="nt_sb")
                    nc.scalar.activation(nt_sb[:], nt_ps[:], ACTF.Copy)
                    nc.sync.dma_start(nt_own[b * P:(b + 1) * P, :],
                                      nt_sb[:])
                    prev = (ehb, u2, sxE, sxO, nt_sb, e0)

            # ---- special (correction) rows ----
            mF_ps = ps_m.tile([P, 4 * P], F32, tag="m")
            nc.tensor.matmul(mF_ps[0:H, 0:P], lhsT=Wmsg2_ap,
                             rhs=ehF_ap, start=True, stop=True)
            mFT_sb = sb.tile([H, P], F32R, tag="mFT_sb")
            nc.vector.tensor_scalar(out=mFT_sb[:], in0=mF_ps[0:H, 0:P],
                                    scalar1=0.0, scalar2=None, op0=ALU.max)
            mV_ps = ps_msg.tile([P, H], F32, tag="msg")
            nc.tensor.matmul(mV_ps[:], lhsT=mFT_sb[:], rhs=Wum_ap,
                             start=True, stop=True)
            mV_sb = sb.tile([P, H], F32R, tag="mV_sb")
            nc.vector.tensor_copy(mV_sb[:], mV_ps[:])
            ntgD_sb = sb.tile([P, H], BF16, tag="ntgD_sb")
            nc.gpsimd.indirect_dma_start(
                out=ntgD_sb[:], out_offset=None, in_=nt_own[:],
                in_offset=bass.IndirectOffsetOnAxis(ap=didx_sb[:, 0:1],
                                                    axis=0),
            )
            ntgD_f = sb.tile([P, H], F32, tag="ntgD_f")
            nc.vector.tensor_copy(ntgD_f[:], ntgD_sb[:])
            spec_ps = ps_msg.tile([P, H], F32, tag="msg")
            nc.tensor.matmul(spec_ps[:], lhsT=Sneg_ap, rhs=mV_sb[:],
                             start=True, stop=True)
            spec_sb = sb.tile([P, H], BF16, tag="spec_sb")
            nc.vector.tensor_tensor(out=spec_sb[:], in0=spec_ps[:],
                                    in1=ntgD_f[:], op=ALU.add)

            # ---- fix-up group for the corrected edges ----
            mf_ps = ps_m.tile([P, 4 * P], F32, tag="m")
            nc.tensor.matmul(mf_ps[0:H, 0:P], lhsT=Wmsg2_ap,
                             rhs=ehRF_ap, start=True, stop=True)
            mfT_sb = sb.tile([H, P], BF16, tag="mrevT")
            nc.scalar.activation(mfT_sb[:], mf_ps[0:H, 0:P], ACTF.Relu)
            of_ps = ps_o.tile([P, 4 * P], F32, tag="o")
            nc.tensor.matmul(of_ps[0:H, 0:P], lhsT=Wua_ap,
                             rhs=attrF_ap, start=True, stop=False)
            nc.tensor.matmul(of_ps[0:H, 0:P], lhsT=negWum_ap,
                             rhs=mfT_sb[:], start=False, stop=False)
            nc.tensor.matmul(of_ps[0:H, 0:P], lhsT=spec_sb[:],
                             rhs=ident_ap, start=False, stop=True)
            outF_sb = sb.tile([H, P], BF16, tag="outF")
            nc.vector.tensor_scalar(out=outF_sb[:], in0=of_ps[0:H, 0:P],
                                    scalar1=0.0, scalar2=None, op0=ALU.max)
            nc.sync.dma_start(outF[:], outF_sb[:])

    nc.compile()
    return nc


def _host_prep(x, edge_attr, edge_hidden, W_msg, b_msg, W_upd, b_upd,
               edge_index):
    src = np.asarray(edge_index[0], dtype=np.int64)
    tgt = np.asarray(edge_index[1], dtype=np.int64)
    eh = np.asarray(edge_hidden, dtype=np.float32)
    ea = np.asarray(edge_attr, dtype=np.float32)
    x = np.asarray(x, dtype=np.float32)
    W_msg = np.asarray(W_msg, dtype=np.float32)
    b_msg = np.asarray(b_msg, dtype=np.float32)
    W_upd = np.asarray(W_upd, dtype=np.float32)
    b_upd = np.asarray(b_upd, dtype=np.float32)
    assert not np.any(b_msg), "nonzero b_msg unsupported by this build"

    # ---- balance nodes into blocks (FFD on in-degree) ----
    deg = np.bincount(tgt, minlength=N)
    k_blk = 20
    packs = []           # per core: (blockof, slotof) or None
    for c in range(NC):
        dc = deg[c * NPC:(c + 1) * NPC]
        order_d = np.argsort(-dc, kind="stable")
        loads = np.zeros(NBLK, np.int64)
        counts = np.zeros(NBLK, np.int64)
        blockof = np.empty(NPC, np.int64)
        slotof = np.empty(NPC, np.int64)
        cap = k_blk * P
        feasible = True
        for v in order_d:
            w = dc[v]
            mask = (counts < P) & (loads + w <= cap)
            if not mask.any():
                feasible = False
                break
            bsel = np.argmin(np.where(mask, loads, np.iinfo(np.int64).max))
            blockof[v] = bsel
            slotof[v] = counts[bsel]
            counts[bsel] += 1
            loads[bsel] += w
        if not feasible:
            break
        packs.append((blockof, slotof))
    if len(packs) < NC:
        # fallback: consecutive 128-node blocks, computed k_blk
        order = np.argsort(tgt, kind="stable")
        tgt_s = tgt[order]
        runs = np.empty((NC, NBLK), np.int64)
        for c in range(NC):
            for b in range(NBLK):
                lo_n = c * NPC + b * P
                hi_n = min(c * NPC + (b + 1) * P, (c + 1) * NPC)
                runs[c, b] = (np.searchsorted(tgt_s, hi_n, "left")
                              - np.searchsorted(tgt_s, lo_n, "left"))
        k_blk = int(np.ceil(runs.max() / P))
        if k_blk % 2:
            k_blk += 1
        packs = []
        for c in range(NC):
            blockof = np.minimum(np.arange(NPC) // P, NBLK - 1)
            slotof = np.arange(NPC) % P
            packs.append((blockof, slotof))
    nch = NBLK * k_blk
    l1 = nch * P
    lblk = k_blk * P
    hb = lblk // 2

    # ---- exclusion groups (reference's int logic) ----
    keys = tgt * N + src
    q = src * N + tgt
    order2 = np.argsort(keys, kind="stable")
    sk = keys[order2]
    lo2 = np.searchsorted(sk, q, "left")
    hi2 = np.searchsorted(sk, q, "right")
    eids = np.arange(E, dtype=np.int64)
    rev = np.where(eids < E2, eids + E2, eids - E2)
    simple = (hi2 - lo2 == 1) & (order2[lo2] == rev)
    affected = np.where(~simple)[0]

    Wmsg_io = np.ascontiguousarray(W_msg.T)         # [in, out]
    Wmsg2 = np.concatenate([Wmsg_io, Wmsg_io], axis=0)
    Z = np.zeros((H, H), np.float32)
    Wpair = np.block([[Wmsg_io, Z], [Z, Wmsg_io]])
    nWum = -W_upd[:, H + A:].T
    negWum2 = np.concatenate([nWum, nWum], axis=0)
    iota1 = np.tile(np.arange(P, dtype=np.float32), (P, 1))
    iotaP = np.arange(P, dtype=np.float32).reshape(P, 1)
    Wstack = np.concatenate(
        [-W_upd[:, H + A:].T, W_upd[:, H:H + A].T], axis=0)
    Wua = np.ascontiguousarray(W_upd[:, H:H + A].T)
    Wum = np.ascontiguousarray(W_upd[:, H + A:].T)
    Wux = np.ascontiguousarray(W_upd[:, :H].T)

    # parity permutation: padded position p (chunk-major) -> device slot
    ch_g = np.arange(l1) // P          # global chunk idx
    kk = np.arange(l1) % P
    i_loc = ch_g % k_blk
    b_of = ch_g // k_blk
    halfsel = (i_loc % 2).astype(np.int64)            # 0 even, 1 odd
    col2 = (ch_g // 2) * P + kk                        # outT2/eh col
    # parity-split order inside a block (for trel_rows / attr / u2):
    ppos = (b_of * lblk + halfsel * hb
            + (i_loc // 2) * P + kk)                   # 0..l1-1

    in_maps = []
    meta = []
    for c in range(NC):
        blockof, slotof = packs[c]
        nodeslot = blockof * P + slotof
        f_idx = np.where((tgt >= c * NPC) & (tgt < (c + 1) * NPC))[0]
        tl = tgt[f_idx] - c * NPC
        eb = blockof[tl]
        es = slotof[tl]
        order_e = np.lexsort((es, eb))
        f_sorted = f_idx[order_e]
        eb_s = eb[order_e]
        es_s = es[order_e]
        gl = np.zeros(l1, np.int64)      # in-edge f per padded position
        trel = np.full(l1, -1.0, np.float32)
        valid = np.zeros(l1, bool)
        starts = np.searchsorted(eb_s, np.arange(NBLK), "left")
        ends = np.searchsorted(eb_s, np.arange(NBLK), "right")
        for b in range(NBLK):
            lo, hi = starts[b], ends[b]
            n = hi - lo
            assert n <= lblk
            base = b * k_blk * P
            gl[base:base + n] = f_sorted[lo:hi]
            trel[base:base + n] = es_s[lo:hi]
            valid[base:base + n] = True

        ehp = eh[gl].astype(NPBF)                     # [l1, 64]
        eh2 = np.empty((P, (nch // 2) * P), NPBF)
        ehpT = ehp.T                                  # [64, l1]
        eh2[0:H, col2[halfsel == 0]] = ehpT[:, halfsel == 0]
        eh2[H:P, col2[halfsel == 1]] = ehpT[:, halfsel == 1]

        trel_nch = trel.reshape(nch, P)               # [ch, 128]
        tgt_rel8 = np.repeat(trel_nch.T, 8, axis=1)
        # parity-split trel, replicated to all partitions (for u2)
        trel_ps = np.empty(l1, np.float32)
        trel_ps[ppos] = trel
        trel_bca = np.broadcast_to(
            trel_ps.astype(NPBF)[None, :], (P, l1))

        # pass 2: out-edge e = rev(f); src_e = tgt_f
        el = rev[gl]
        attr_full = ea[el]                            # [l1, A]
        attr_ps = np.empty((l1, A), np.float32)
        attr_ps[ppos] = attr_full
        attr_T = np.ascontiguousarray(
            attr_ps.reshape(NBLK, lblk, A).transpose(0, 2, 1)
        ).reshape(NBLK, A, lblk)
        # flatten to [A, l1] with per-block contiguous regions
        attr_Tc = np.concatenate([attr_T[b] for b in range(NBLK)],
                                 axis=1).astype(NPBF)

        xpad = np.zeros((NPC_PAD, H + 1), np.float32)
        xpad[nodeslot, :H] = x[c * NPC:(c + 1) * NPC]
        xpad[:, H] = 1.0

        # corrections
        aff_c = affected[(src[affected] >= c * NPC)
                         & (src[affected] < (c + 1) * NPC)]
        f_list, s_cols = [], []
        for d, e in enumerate(aff_c):
            for f in order2[lo2[e]:hi2[e]]:
                if f != rev[e]:
                    f_list.append(f)
                    s_cols.append(d)
        assert len(aff_c) <= SPEC_CAP, len(aff_c)
        assert len(f_list) <= P, len(f_list)
        ehF = np.zeros((P, H), np.float32)
        if f_list:
            ehF[:len(f_list)] = eh[np.asarray(f_list)]
        ehRF = np.zeros((P, H), np.float32)
        attrF = np.zeros((P, A), np.float32)
        if len(aff_c):
            ehRF[:len(aff_c)] = eh[rev[aff_c]]
            attrF[:len(aff_c)] = ea[aff_c]
        Sneg = np.zeros((P, P), np.float32)
        for fi, d in enumerate(s_cols):
            Sneg[fi, d] = -1.0
        didx = np.zeros((P, 1), np.int32)
        didx[:len(aff_c), 0] = nodeslot[src[aff_c] - c * NPC]

        pB = np.zeros((P, PB_COLS), np.float32)
        pB[:, PB_WPAIR:PB_WPAIR + P] = Wpair
        pB[:, PB_WMSG2:PB_WMSG2 + H] = Wmsg2
        pB[:, PB_NWUM2:PB_NWUM2 + H] = negWum2
        pB[0:H + A, PB_WSTACK:PB_WSTACK + H] = Wstack
        pB[0:A, PB_WUA:PB_WUA + H] = Wua
        pB[:, PB_IOTA1:PB_IOTA1 + P] = iota1
        pB[:, PB_IDENT:PB_IDENT + P] = np.eye(P, dtype=np.float32)
        pB[0:H, PB_EHF:PB_EHF + P] = ehF.T
        pB[0:H, PB_EHRF:PB_EHRF + P] = ehRF.T
        pB[0:A, PB_ATTRF:PB_ATTRF + P] = attrF.T
        pR = np.zeros((P, PR_COLS), np.float32)
        pR[0:H, PR_WUM2:PR_WUM2 + H] = Wum
        pR[H:P, PR_WUM2:PR_WUM2 + H] = Wum
        pR[0:H, PR_WUXB:PR_WUXB + H] = Wux
        pR[H, PR_WUXB:PR_WUXB + H] = b_upd
        pR[:, PR_SNEG:PR_SNEG + P] = Sneg
        pF = np.zeros((P, PF_COLS), np.float32)
        pF[:, PF_IOTAP:PF_IOTAP + 1] = iotaP

        in_maps.append({
            "eh2": eh2,
            "tgt_rel8": np.ascontiguousarray(tgt_rel8).astype(NPBF),
            "trel_bca": np.ascontiguousarray(trel_bca),
            "attr_T": attr_Tc,
            "xT_own": np.ascontiguousarray(xpad.T),
            "packB": pB.astype(NPBF),
            "packR": pR,
            "packF": pF,
            "didx": didx,
        })
        meta.append({"el": el, "valid": valid, "aff_c": aff_c,
                     "halfsel": halfsel, "col2": col2})
    return in_maps, meta, k_blk


def kernel(**inputs) -> np.ndarray:
    in_maps, meta, k_blk = _host_prep(**inputs)
    if k_blk not in _CACHE:
        _CACHE[k_blk] = _build(k_blk)
    nc = _CACHE[k_blk]
    res = run_bass_kernel_spmd(nc, in_maps, core_ids=list(range(NC)))
    l1 = NBLK * k_blk * P
    out = np.empty((E, H), np.float32)
    for c in range(NC):
        m = meta[c]
        oT2 = np.asarray(res.results[c]["outT2"], dtype=np.float32)
        vals = np.empty((l1, H), np.float32)
        sel = m["halfsel"] == 0
        vals[sel] = oT2[0:H, m["col2"][sel]].T
        vals[~sel] = oT2[H:P, m["col2"][~sel]].T
        out[m["el"][m["valid"]]] = vals[m["valid"]]
    for c in range(NC):
        aff_c = meta[c]["aff_c"]
        if len(aff_c):
            oF = np.asarray(res.results[c]["outF"], dtype=np.float32)
            out[aff_c] = oF[:, :len(aff_c)].T
    return out
